# revision 5
# baseline (speedup 1.0000x reference)
"""Trainium2 Bass kernel for nn_Loss4PixelReconstruction.

reference: recon = sum_k shift_k(image1) * filters[k]  (11x11 dynamic
per-pixel filter, shared across RGB), loss = mean(sqrt((recon-image2)^2+eps^2)).

Sharding: data-parallel over (N=4) x (H split in 2) -> 8 cores.
Each core: local Charbonnier partial sum; host sums the 8 scalars.
"""

import sys

sys.path.insert(0, "/opt/trn_rl_repo")

import numpy as np

K = 11
PAD = 5
EPS = 1e-3
N, C, H, W = 4, 3, 256, 256
HSH = 128          # output rows per core
IMG_H = HSH + 2 * PAD   # 138 padded input rows per core
IMG_W = W + 2 * PAD     # 266 padded input cols

_CACHE = {}
LAST_RESULTS = None


def _build_nc():
    import concourse.bass as bass
    import concourse.tile as tile
    from concourse import bacc, mybir
    from concourse import bass_isa
    from contextlib import ExitStack

    f32 = mybir.dt.float32
    nc = bacc.Bacc("TRN2", target_bir_lowering=False, debug=False)

    img1p = nc.declare_dram_parameter("img1p", [C, IMG_H, IMG_W], f32, isOutput=False)
    img2 = nc.declare_dram_parameter("img2", [C, HSH, W], f32, isOutput=False)
    flt = nc.declare_dram_parameter("flt", [K * K, HSH, W], f32, isOutput=False)
    out = nc.declare_dram_parameter("out", [1, 1], f32, isOutput=True)

    with ExitStack() as ctx:
        tc = ctx.enter_context(tile.TileContext(nc))
        imgsh_pool = ctx.enter_context(tc.tile_pool(name="imgsh", bufs=1))
        fpool = ctx.enter_context(tc.tile_pool(name="fchunk", bufs=2))
        accp = ctx.enter_context(tc.tile_pool(name="acc", bufs=1))
        tmpp = ctx.enter_context(tc.tile_pool(name="tmp", bufs=4))
        redp = ctx.enter_context(tc.tile_pool(name="red", bufs=1))

        # 11 dy-shifted copies of the padded image: imgsh[h, dy, c, x] = img1p[c, h+dy, x]
        imgsh = imgsh_pool.tile([HSH, K, C, IMG_W], f32)
        for dy in range(K):
            nc.sync.dma_start(
                imgsh[:, dy, :, :],
                img1p[:, dy:dy + HSH, :].rearrange("c h w -> h c w"),
            )

        facc = accp.tile([HSH, C, W], f32)

        # stream filters one dy-row (11 taps) at a time
        for dy in range(K):
            fch = fpool.tile([HSH, K, W], f32)
            nc.sync.dma_start(
                fch[:], flt[dy * K:(dy + 1) * K, :, :].rearrange("k h w -> h k w")
            )
            for dx in range(K):
                for c in range(C):
                    src_img = imgsh[:, dy, c, dx:dx + W]
                    fsl = fch[:, dx, :]
                    if dy == 0 and dx == 0:
                        nc.vector.tensor_tensor(
                            facc[:, c, :], src_img, fsl, mybir.AluOpType.mult
                        )
                    else:
                        t = tmpp.tile([HSH, W], f32, tag="prod")
                        nc.vector.tensor_tensor(
                            t[:], src_img, fsl, mybir.AluOpType.mult
                        )
                        nc.vector.tensor_add(facc[:, c, :], facc[:, c, :], t[:])

        # Charbonnier: sqrt((recon - img2)^2 + eps^2), summed
        img2t = redp.tile([HSH, C, W], f32)
        nc.sync.dma_start(img2t[:], img2[:, :, :].rearrange("c h w -> h c w"))
        diff = redp.tile([HSH, C, W], f32)
        nc.vector.tensor_tensor(diff[:], facc[:], img2t[:], mybir.AluOpType.subtract)
        d2 = redp.tile([HSH, C, W], f32)
        nc.vector.tensor_tensor(d2[:], diff[:], diff[:], mybir.AluOpType.mult)
        charb = redp.tile([HSH, C, W], f32)
        rowsum = redp.tile([HSH, 1], f32)
        eps2 = redp.tile([HSH, 1], f32)
        nc.vector.memset(eps2[:], EPS * EPS)
        nc.scalar.activation(
            charb[:], d2[:], mybir.ActivationFunctionType.Sqrt,
            bias=eps2[:], scale=1.0, accum_out=rowsum[:],
        )
        total = redp.tile([HSH, 1], f32)
        nc.gpsimd.partition_all_reduce(
            total[:], rowsum[:], channels=HSH, reduce_op=bass_isa.ReduceOp.add
        )
        nc.sync.dma_start(out[:, :], total[0:1, :])

    nc.compile()
    return nc


def _get_nc():
    if "nc" not in _CACHE:
        _CACHE["nc"] = _build_nc()
    return _CACHE["nc"]


def _shard_inputs(image1, image2, filters):
    in_maps = []
    for core in range(8):
        n, hb = core // 2, core % 2
        h0 = hb * HSH
        img1p = np.zeros((C, IMG_H, IMG_W), np.float32)
        lo = max(0, h0 - PAD)
        hi = min(H, h0 + HSH + PAD)
        img1p[:, lo - (h0 - PAD):lo - (h0 - PAD) + (hi - lo), PAD:PAD + W] = \
            image1[n, :, lo:hi, :]
        in_maps.append({
            "img1p": img1p,
            "img2": np.ascontiguousarray(image2[n, :, h0:h0 + HSH, :]),
            "flt": np.ascontiguousarray(filters[n, :, h0:h0 + HSH, :]),
        })
    return in_maps


def kernel(image1, image2, filters):
    global LAST_RESULTS
    import os
    from concourse.bass_utils import run_bass_kernel_spmd

    nc = _get_nc()
    in_maps = _shard_inputs(
        np.asarray(image1, np.float32),
        np.asarray(image2, np.float32),
        np.asarray(filters, np.float32),
    )
    trace = bool(int(os.environ.get("KERNEL_TRACE", "0")))
    res = run_bass_kernel_spmd(nc, in_maps, list(range(8)), trace=trace)
    LAST_RESULTS = res
    parts = [float(res.results[i]["out"][0, 0]) for i in range(8)]
    return np.float32(sum(parts) / (N * C * H * W))


# revision 6
# speedup vs baseline: 1.9713x; 1.9713x over previous
"""Trainium2 Bass kernel for nn_Loss4PixelReconstruction.

reference: recon = sum_k shift_k(image1) * filters[k]  (11x11 dynamic
per-pixel filter, shared across RGB), loss = mean(sqrt((recon-image2)^2+eps^2)).

Sharding: data-parallel over (N=4) x (H split in 2) -> 8 cores.
Each core: local Charbonnier partial sum; host sums the 8 scalars.

v1: bf16 compute on DVE (2x tensor_tensor mode), ACT does fp32->bf16
conversions + Charbonnier sqrt with fused row-sum accumulation.
Layout: partition = h (128 rows/core), free = (c, w). Per-tap (dy,dx)
multiply uses dy-shifted bf16 image copies (even- and odd-offset copies
keep the 4B alignment needed for the DVE 2x packed mode); products for
one dy-row (11 taps) are tree-added, then accumulated across dy.
"""

import sys

sys.path.insert(0, "/opt/trn_rl_repo")

import numpy as np

K = 11
PAD = 5
EPS = 1e-3
N, C, H, W = 4, 3, 256, 256
HSH = 128               # output rows per core
IMG_H = HSH + 2 * PAD   # 138 padded input rows per core
W_PAD = 268             # padded input cols (5 + 256 + 7)
CW = C * W

_CACHE = {}
LAST_RESULTS = None


def _build_nc():
    import concourse.tile as tile
    from concourse import bacc, mybir
    from concourse import bass_isa
    from contextlib import ExitStack

    f32 = mybir.dt.float32
    bf16 = mybir.dt.bfloat16
    MUL = mybir.AluOpType.mult
    ADD = mybir.AluOpType.add
    SUB = mybir.AluOpType.subtract

    nc = bacc.Bacc("TRN2", target_bir_lowering=False, debug=False)

    img1p = nc.declare_dram_parameter("img1p", [C, IMG_H, W_PAD], f32, isOutput=False)
    img2 = nc.declare_dram_parameter("img2", [C, HSH, W], f32, isOutput=False)
    flt = nc.declare_dram_parameter("flt", [K * K, HSH, W], f32, isOutput=False)
    out = nc.declare_dram_parameter("out", [1, 1], f32, isOutput=True)

    with ExitStack() as ctx:
        tc = ctx.enter_context(tile.TileContext(nc))
        stagep = ctx.enter_context(tc.tile_pool(name="stage", bufs=3))
        imp = ctx.enter_context(tc.tile_pool(name="im", bufs=1))
        ffpp = ctx.enter_context(tc.tile_pool(name="ffp", bufs=2))
        fbfp = ctx.enter_context(tc.tile_pool(name="fbf", bufs=2))
        prodp = ctx.enter_context(tc.tile_pool(name="prod", bufs=2))
        trep = ctx.enter_context(tc.tile_pool(name="tre", bufs=2))
        accp = ctx.enter_context(tc.tile_pool(name="acc", bufs=1))
        tailp = ctx.enter_context(tc.tile_pool(name="tail", bufs=1))

        # dy-shifted bf16 image copies; _o is shifted one more column so
        # odd-dx taps read at 4B-aligned offsets.
        ime = imp.tile([HSH, K, C, W_PAD], bf16)
        imo = imp.tile([HSH, K, C, W_PAD], bf16)
        for dy in range(K):
            st = stagep.tile([HSH, C, W_PAD], f32, tag="stage")
            nc.sync.dma_start(
                st[:], img1p[:, dy:dy + HSH, :].rearrange("c h w -> h c w")
            )
            nc.scalar.copy(ime[:, dy, :, :], st[:])
            nc.scalar.copy(imo[:, dy, :, 0:W_PAD - 2], st[:, :, 1:W_PAD - 1])

        dyacc = accp.tile([HSH, CW], bf16)

        for dy in range(K):
            ff = ffpp.tile([HSH, K, W], f32)
            nc.sync.dma_start(
                ff[:], flt[dy * K:(dy + 1) * K, :, :].rearrange("k h w -> h k w")
            )
            fb = fbfp.tile([HSH, K, W], bf16)
            nc.scalar.copy(fb[:], ff[:])

            pr = prodp.tile([HSH, K, CW], bf16)
            for dx in range(K):
                if dx % 2 == 0:
                    src = ime[:, dy, :, dx:dx + W]
                else:
                    src = imo[:, dy, :, dx - 1:dx - 1 + W]
                fbc = fb[:, dx:dx + 1, :].broadcast_to([HSH, C, W])
                dst = pr[:, dx, :].rearrange("p (c w) -> p c w", c=C)
                nc.vector.tensor_tensor(dst, src, fbc, MUL)

            # tree-reduce the 11 product planes
            pr10 = pr[:, 0:10, :].rearrange("p (t j) cw -> p t j cw", j=2)
            t1 = trep.tile([HSH, 5, CW], bf16, tag="t1")
            nc.vector.tensor_tensor(
                t1[:], pr10[:, :, 0, :], pr10[:, :, 1, :], ADD
            )
            t14 = t1[:, 0:4, :].rearrange("p (t j) cw -> p t j cw", j=2)
            t2 = trep.tile([HSH, 2, CW], bf16, tag="t2")
            nc.vector.tensor_tensor(t2[:], t14[:, :, 0, :], t14[:, :, 1, :], ADD)
            tA = trep.tile([HSH, CW], bf16, tag="tA")
            nc.vector.tensor_tensor(tA[:], t2[:, 0, :], t2[:, 1, :], ADD)
            tB = trep.tile([HSH, CW], bf16, tag="tB")
            nc.vector.tensor_tensor(tB[:], t1[:, 4, :], pr[:, 10, :], ADD)
            if dy == 0:
                nc.vector.tensor_tensor(dyacc[:], tA[:], tB[:], ADD)
            else:
                tC = trep.tile([HSH, CW], bf16, tag="tC")
                nc.vector.tensor_tensor(tC[:], tA[:], tB[:], ADD)
                nc.vector.tensor_tensor(dyacc[:], dyacc[:], tC[:], ADD)

        # Charbonnier tail
        i2s = tailp.tile([HSH, C, W], f32)
        nc.sync.dma_start(i2s[:], img2[:, :, :].rearrange("c h w -> h c w"))
        i2b = tailp.tile([HSH, CW], bf16)
        nc.scalar.copy(i2b[:], i2s[:].rearrange("p c w -> p (c w)"))
        diff = tailp.tile([HSH, CW], bf16)
        nc.vector.tensor_tensor(diff[:], dyacc[:], i2b[:], SUB)
        d2 = tailp.tile([HSH, CW], bf16)
        nc.vector.tensor_tensor(d2[:], diff[:], diff[:], MUL)
        charb = tailp.tile([HSH, CW], f32)
        rowsum = tailp.tile([HSH, 1], f32)
        eps2 = tailp.tile([HSH, 1], f32)
        nc.vector.memset(eps2[:], EPS * EPS)
        nc.scalar.activation(
            charb[:], d2[:], mybir.ActivationFunctionType.Sqrt,
            bias=eps2[:], scale=1.0, accum_out=rowsum[:],
        )
        total = tailp.tile([HSH, 1], f32)
        nc.gpsimd.partition_all_reduce(
            total[:], rowsum[:], channels=HSH, reduce_op=bass_isa.ReduceOp.add
        )
        nc.sync.dma_start(out[:, :], total[0:1, :])

    nc.compile()
    return nc


def _get_nc():
    if "nc" not in _CACHE:
        _CACHE["nc"] = _build_nc()
    return _CACHE["nc"]


def _shard_inputs(image1, image2, filters):
    in_maps = []
    for core in range(8):
        n, hb = core // 2, core % 2
        h0 = hb * HSH
        img1p = np.zeros((C, IMG_H, W_PAD), np.float32)
        lo = max(0, h0 - PAD)
        hi = min(H, h0 + HSH + PAD)
        img1p[:, lo - (h0 - PAD):lo - (h0 - PAD) + (hi - lo), PAD:PAD + W] = \
            image1[n, :, lo:hi, :]
        in_maps.append({
            "img1p": img1p,
            "img2": np.ascontiguousarray(image2[n, :, h0:h0 + HSH, :]),
            "flt": np.ascontiguousarray(filters[n, :, h0:h0 + HSH, :]),
        })
    return in_maps


def kernel(image1, image2, filters):
    global LAST_RESULTS
    import os
    from concourse.bass_utils import run_bass_kernel_spmd

    nc = _get_nc()
    in_maps = _shard_inputs(
        np.asarray(image1, np.float32),
        np.asarray(image2, np.float32),
        np.asarray(filters, np.float32),
    )
    trace = bool(int(os.environ.get("KERNEL_TRACE", "0")))
    res = run_bass_kernel_spmd(nc, in_maps, list(range(8)), trace=trace)
    LAST_RESULTS = res
    parts = [float(res.results[i]["out"][0, 0]) for i in range(8)]
    return np.float32(sum(parts) / (N * C * H * W))
